# revision 6
# baseline (speedup 1.0000x reference)
"""Multi-head causal self-attention (B=2, S=2048, D=2048, H=16) on 8 trn2
NeuronCores.

Sharding: tensor-parallel over heads. Core c owns heads {2c, 2c+1}:
  - QKV projection for its 2 heads (contraction over the full d_model),
  - causal attention for its 2 heads,
  - partial output projection  O_c = A_c @ W_out[:, c*256:(c+1)*256].T
Host sums the 8 partial outputs (the "all-reduce after out_proj" of the
TP scheme, done on host since the full output is assembled there anyway).

All on-device compute is laid out "feature-major" (transposed) so no
transposes are ever needed:
  - x is shipped as xT [D, B*S]
  - Q^T, K^T per head as [Dh=128, S];  V token-major as [S, Dh] tiles
  - scores are built transposed: S^T[k, q] = (K Q^T)[k, q]
  - softmax without max-subtraction (scores are O(+-5)), with the
    normalizer computed by a ones-row matmul over partition (key) dim and
    applied via a rank-1 PE broadcast.
  - attention output lands as A^T [Dh, S]; out-proj consumes it directly.

Matmuls run as float32r (full PE rate at free-dim >= 256, fp32 storage).
"""

import math

import numpy as np

import concourse.bass as bass
import concourse.tile as tile
from concourse import bacc, mybir
from concourse.bass_utils import run_bass_kernel_spmd

F32 = mybir.dt.float32
F32R = mybir.dt.float32r

N_CORES = 8


class Cfg:
    def __init__(self, B=2, S=2048, D=2048, n_heads=16):
        self.B = B
        self.S = S
        self.D = D
        self.n_heads = n_heads
        self.Dh = 128
        self.DHT = n_heads * self.Dh       # W_qkv section stride (q/k/v)
        self.HPC = n_heads // N_CORES      # heads per core (2)
        self.QC = 512                      # token chunk (matmul free dim)
        self.KT = D // 128                 # k-tiles over d_model
        self.NCH = S // self.QC            # token chunks per batch
        assert self.HPC == 2 and D % 128 == 0 and S % self.QC == 0


def build_kernel(cfg: Cfg):
    """Build the SPMD single-core program. Returns compiled nc."""
    B, S, D, QC, KT, NCH = cfg.B, cfg.S, cfg.D, cfg.QC, cfg.KT, cfg.NCH
    Dh = cfg.Dh
    NQT = QC // 128                      # 128-token subtiles per chunk
    inv_sqrt_dh = 1.0 / math.sqrt(Dh)

    nc = bacc.Bacc("TRN2", target_bir_lowering=False, debug=False,
                   num_devices=N_CORES)

    xT = nc.dram_tensor("xT", [D, B * S], F32R, kind="ExternalInput").ap()
    wqkvT = nc.dram_tensor("wqkvT", [D, 768], F32R, kind="ExternalInput").ap()
    woutT = nc.dram_tensor("woutT", [256, D], F32R, kind="ExternalInput").ap()
    masks = nc.dram_tensor("masks", [128, NQT * QC], F32R,
                           kind="ExternalInput").ap()
    ones_col = nc.dram_tensor("ones_col", [128, 1], F32R,
                              kind="ExternalInput").ap()
    ones_row = nc.dram_tensor("ones_row", [1, 128], F32R,
                              kind="ExternalInput").ap()
    outT = nc.dram_tensor("outT", [D, B * S], F32, kind="ExternalOutput").ap()


    with tile.TileContext(nc) as tc:
        with (
            tc.tile_pool(name="wpool", bufs=1) as wpool,
            tc.tile_pool(name="xpool", bufs=18) as xpool,
            tc.tile_pool(name="qkvpool", bufs=1) as qkvpool,
            tc.tile_pool(name="apool", bufs=1) as apool,
            tc.tile_pool(name="ppool", bufs=4) as ppool,
            tc.tile_pool(name="opool", bufs=3) as opool,
            tc.tile_pool(name="smallpool", bufs=2) as smallpool,
            tc.tile_pool(name="pspool", bufs=4, space="PSUM") as pspool,
            tc.tile_pool(name="attnps", bufs=2, space="PSUM") as attnps,
            tc.tile_pool(name="rps", bufs=2, space="PSUM") as rps,
        ):
            # ---- static weights / constants ----
            w_tiles = []
            for k in range(KT):
                t = wpool.tile([128, 768], F32R, tag=f"w{k}", name=f"w{k}")
                nc.sync.dma_start(t[:], wqkvT[k * 128:(k + 1) * 128, :])
                w_tiles.append(t)
            wo_tiles = []
            for hh in range(2):
                t = wpool.tile([128, D], F32R, tag=f"wo{hh}", name=f"wo{hh}")
                nc.sync.dma_start(t[:], woutT[hh * 128:(hh + 1) * 128, :])
                wo_tiles.append(t)
            mask_t = wpool.tile([128, NQT * QC], F32R, tag="mask", name="mask")
            nc.sync.dma_start(mask_t[:], masks[:])
            onec_t = wpool.tile([128, 1], F32R, tag="onec", name="onec")
            nc.sync.dma_start(onec_t[:], ones_col[:])
            oner_t = wpool.tile([1, 128], F32R, tag="oner", name="oner")
            nc.sync.dma_start(oner_t[:], ones_row[:])

            for b in range(B):
                # ---- persistent per-batch QKV / A tiles ----
                # comps: 0=Q_h0 1=K_h0 2=Q_h1 3=K_h1 (dh-major [128, S])
                qk_sb = [qkvpool.tile([128, S], F32R, tag=f"qk{c}", name=f"qk{c}")
                         for c in range(4)]
                # V token-major: tile per 128 tokens, [128, 256] (2 heads)
                v_sb = [qkvpool.tile([128, 256], F32R, tag=f"v{t}", name=f"v{t}")
                        for t in range(S // 128)]
                # A^T per head [128, S]
                a_sb = [apool.tile([128, S], F32R, tag=f"a{h}", name=f"a{h}")
                        for h in range(2)]

                # ======== Phase A: QKV projection for this batch ========
                for j in range(NCH):
                    col0 = b * S + j * QC
                    xt = []
                    for k in range(KT):
                        t = xpool.tile([128, QC], F32R, tag="xt", name="xt")
                        nc.sync.dma_start(
                            t[:], xT[k * 128:(k + 1) * 128, col0:col0 + QC])
                        xt.append(t)
                    # Q^T / K^T for both heads
                    for c in range(4):
                        ps = pspool.tile([128, QC], F32, tag="ps", name="ps")
                        for k in range(KT):
                            nc.tensor.matmul(
                                ps[:],
                                (w_tiles[k][:, c * 128:(c + 1) * 128]),
                                (xt[k][:]),
                                start=(k == 0), stop=(k == KT - 1))
                        nc.vector.tensor_copy(
                            qk_sb[c][:, j * QC:(j + 1) * QC], ps[:])
                    # V token-major (both heads side by side)
                    for sub in range(NQT):
                        ps = pspool.tile([128, 256], F32, tag="ps", name="ps")
                        for k in range(KT):
                            nc.tensor.matmul(
                                ps[:],
                                (xt[k][:, sub * 128:(sub + 1) * 128]),
                                (w_tiles[k][:, 512:768]),
                                start=(k == 0), stop=(k == KT - 1))
                        nc.vector.tensor_copy(v_sb[j * NQT + sub][:], ps[:])

                # ======== Phase B+C: attention + out-proj per chunk ======
                for j in range(NCH):
                    n_kt = (j + 1) * QC // 128
                    for h in range(2):
                        qT = qk_sb[2 * h]
                        kT = qk_sb[2 * h + 1]
                        attn = attnps.tile([128, QC], F32, tag="attn", name="attn")
                        r = rps.tile([1, QC], F32, tag="r", name="r")
                        for kt in range(n_kt):
                            s_ps = pspool.tile([128, QC], F32, tag="ps", name="ps")
                            nc.tensor.matmul(
                                s_ps[:],
                                (kT[:, kt * 128:(kt + 1) * 128]),
                                (qT[:, j * QC:(j + 1) * QC]),
                                start=True, stop=True)
                            p_sb = ppool.tile([128, QC], F32R, tag="p", name="p")
                            nc.scalar.activation(
                                p_sb[:], s_ps[:],
                                mybir.ActivationFunctionType.Exp,
                                scale=inv_sqrt_dh)
                            rel = kt * 128 - j * QC
                            if rel >= 0:
                                # diagonal block: zero the k > q half
                                ridx = rel // 128
                                nc.vector.tensor_mul(
                                    p_sb[:], p_sb[:],
                                    mask_t[:, ridx * QC:(ridx + 1) * QC])
                            nc.tensor.matmul(
                                attn[:],
                                (v_sb[kt][:, h * 128:(h + 1) * 128]),
                                (p_sb[:]),
                                start=(kt == 0), stop=(kt == n_kt - 1))
                            nc.tensor.matmul(
                                r[:], (onec_t[:]), (p_sb[:]),
                                start=(kt == 0), stop=(kt == n_kt - 1))
                        recip = smallpool.tile([1, QC], F32, tag="recip", name="recip")
                        nc.vector.reciprocal(recip[:], r[:])
                        recip_r = smallpool.tile([1, QC], F32R, tag="recipr",
                                                 name="recipr")
                        nc.vector.tensor_copy(recip_r[:], recip[:])
                        rb_ps = pspool.tile([128, QC], F32, tag="ps", name="ps")
                        nc.tensor.matmul(rb_ps[:], (oner_t[:]),
                                         (recip_r[:]), start=True, stop=True)
                        rb_sb = ppool.tile([128, QC], F32R, tag="p", name="p")
                        nc.vector.tensor_copy(rb_sb[:], rb_ps[:])
                        nc.vector.tensor_mul(
                            a_sb[h][:, j * QC:(j + 1) * QC],
                            attn[:], rb_sb[:])
                    # out-proj for this chunk (partial over this core's
                    # 256 head-features)
                    col0 = b * S + j * QC
                    for m in range(D // 128):
                        ps = pspool.tile([128, QC], F32, tag="ps", name="ps")
                        for h in range(2):
                            nc.tensor.matmul(
                                ps[:],
                                (wo_tiles[h][:, m * 128:(m + 1) * 128]),
                                (a_sb[h][:, j * QC:(j + 1) * QC]),
                                start=(h == 0), stop=(h == 1))
                        o_sb = opool.tile([128, QC], F32, tag="o", name="o")
                        nc.vector.tensor_copy(o_sb[:], ps[:])
                        nc.sync.dma_start(
                            outT[m * 128:(m + 1) * 128, col0:col0 + QC],
                            o_sb[:])

    nc.compile()
    return nc


def make_inputs(cfg: Cfg, x, W_qkv, W_out):
    """Host-side sharding: returns in_maps (list of 8 dicts)."""
    B, S, D = cfg.B, cfg.S, cfg.D
    Dh, QC, NQT = cfg.Dh, cfg.QC, cfg.QC // 128
    xTa = np.ascontiguousarray(
        x.reshape(B * S, D).T.astype(np.float32))          # [D, B*S]

    masks = np.zeros((128, NQT * QC), dtype=np.float32)
    for ridx in range(NQT):
        rel = ridx * 128
        p = np.arange(128)[:, None]
        f = np.arange(QC)[None, :]
        masks[:, ridx * QC:(ridx + 1) * QC] = (p + rel <= f).astype(np.float32)
    ones_col = np.ones((128, 1), dtype=np.float32)
    ones_row = np.ones((1, 128), dtype=np.float32)

    in_maps = []
    DHT = cfg.DHT
    for c in range(N_CORES):
        h0 = cfg.HPC * c
        wq = np.empty((D, 768), dtype=np.float32)          # [D, cols]
        for i, h in enumerate((h0, h0 + 1)):
            wq[:, (2 * i) * 128:(2 * i) * 128 + 128] = \
                W_qkv[0 * DHT + h * Dh: 0 * DHT + h * Dh + Dh, :].T   # Q_h
            wq[:, (2 * i + 1) * 128:(2 * i + 1) * 128 + 128] = \
                W_qkv[1 * DHT + h * Dh: 1 * DHT + h * Dh + Dh, :].T   # K_h
            wq[:, 512 + i * 128: 512 + (i + 1) * 128] = \
                W_qkv[2 * DHT + h * Dh: 2 * DHT + h * Dh + Dh, :].T   # V_h
        wo = np.ascontiguousarray(
            W_out[:, h0 * Dh:(h0 + cfg.HPC) * Dh].T.astype(np.float32))
        in_maps.append({
            "xT": xTa,
            "wqkvT": np.ascontiguousarray(wq),
            "woutT": wo,
            "masks": masks,
            "ones_col": ones_col,
            "ones_row": ones_row,
        })
    return in_maps


_CACHED = {}


def kernel(x, W_qkv, W_out, mask=None, **_ignored):
    cfg = Cfg(B=x.shape[0], S=x.shape[1], D=x.shape[2],
              n_heads=W_qkv.shape[0] // 384)
    key = (cfg.B, cfg.S, cfg.D)
    if key not in _CACHED:
        _CACHED[key] = build_kernel(cfg)
    nc = _CACHED[key]
    in_maps = make_inputs(cfg, np.asarray(x), np.asarray(W_qkv),
                          np.asarray(W_out))
    res = run_bass_kernel_spmd(nc, in_maps, list(range(N_CORES)))
    acc = res.results[0]["outT"].astype(np.float32)
    for c in range(1, N_CORES):
        acc = acc + res.results[c]["outT"]
    out = acc.T.reshape(cfg.B, cfg.S, cfg.D)
    return np.ascontiguousarray(out)


# revision 8
# speedup vs baseline: 1.1948x; 1.1948x over previous
"""Multi-head causal self-attention (B=2, S=2048, D=2048, H=16) on 8 trn2
NeuronCores.

Sharding: tensor-parallel over heads. Core c owns heads {2c, 2c+1}:
  - QKV projection for its 2 heads (contraction over the full d_model),
  - causal attention for its 2 heads,
  - partial output projection  O_c = A_c @ W_out[:, c*256:(c+1)*256].T
Host sums the 8 partial outputs (the "all-reduce after out_proj" of the
TP scheme, done on host since the full output is assembled there anyway).

All on-device compute is laid out "feature-major" (transposed) so no
transposes are ever needed:
  - x is shipped as xT [D, B*S]
  - Q^T, K^T per head as [Dh=128, S];  V token-major as [S, Dh] tiles
  - scores are built transposed: S^T[k, q] = (K Q^T)[k, q]
  - softmax without max-subtraction (scores are O(+-5)), with the
    normalizer computed by a ones-row matmul over partition (key) dim and
    applied via a rank-1 PE broadcast.
  - attention output lands as A^T [Dh, S]; out-proj consumes it directly.

Matmuls run as float32r (full PE rate at free-dim >= 256, fp32 storage).
"""

import math

import numpy as np

import concourse.bass as bass
import concourse.tile as tile
from concourse import bacc, mybir
from concourse.bass_utils import run_bass_kernel_spmd

F32 = mybir.dt.float32
F32R = mybir.dt.float32r

N_CORES = 8


class Cfg:
    def __init__(self, B=2, S=2048, D=2048, n_heads=16):
        self.B = B
        self.S = S
        self.D = D
        self.n_heads = n_heads
        self.Dh = 128
        self.DHT = n_heads * self.Dh       # W_qkv section stride (q/k/v)
        self.HPC = n_heads // N_CORES      # heads per core (2)
        self.QC = 512                      # token chunk (matmul free dim)
        self.KT = D // 128                 # k-tiles over d_model
        self.NCH = S // self.QC            # token chunks per batch
        assert self.HPC == 2 and D % 128 == 0 and S % self.QC == 0


def build_kernel(cfg: Cfg):
    """Build the SPMD single-core program. Returns compiled nc."""
    B, S, D, QC, KT, NCH = cfg.B, cfg.S, cfg.D, cfg.QC, cfg.KT, cfg.NCH
    Dh = cfg.Dh
    NQT = QC // 128                      # 128-token subtiles per chunk
    inv_sqrt_dh = 1.0 / math.sqrt(Dh)

    nc = bacc.Bacc("TRN2", target_bir_lowering=False, debug=False,
                   num_devices=N_CORES)

    xT = nc.dram_tensor("xT", [D, B * S], F32R, kind="ExternalInput").ap()
    wqkvT = nc.dram_tensor("wqkvT", [D, 768], F32R, kind="ExternalInput").ap()
    woutT = nc.dram_tensor("woutT", [256, D], F32R, kind="ExternalInput").ap()
    masks = nc.dram_tensor("masks", [128, NQT * QC], F32R,
                           kind="ExternalInput").ap()
    ones_col = nc.dram_tensor("ones_col", [128, 1], F32R,
                              kind="ExternalInput").ap()
    ones_row = nc.dram_tensor("ones_row", [1, 128], F32R,
                              kind="ExternalInput").ap()
    outT = nc.dram_tensor("outT", [D, B * S], F32, kind="ExternalOutput").ap()


    with tile.TileContext(nc) as tc:
        with (
            tc.tile_pool(name="wpool", bufs=1) as wpool,
            tc.tile_pool(name="xpool", bufs=18) as xpool,
            tc.tile_pool(name="qkvpool", bufs=1) as qkvpool,
            tc.tile_pool(name="apool", bufs=1) as apool,
            tc.tile_pool(name="ppool", bufs=4) as ppool,
            tc.tile_pool(name="opool", bufs=3) as opool,
            tc.tile_pool(name="smallpool", bufs=2) as smallpool,
            tc.tile_pool(name="pspool", bufs=4, space="PSUM") as pspool,
            tc.tile_pool(name="attnps", bufs=2, space="PSUM") as attnps,
            tc.tile_pool(name="rps", bufs=2, space="PSUM") as rps,
        ):
            # ---- static weights / constants ----
            w_tiles = []
            for k in range(KT):
                t = wpool.tile([128, 768], F32R, tag=f"w{k}", name=f"w{k}")
                nc.sync.dma_start(t[:], wqkvT[k * 128:(k + 1) * 128, :])
                w_tiles.append(t)
            wo_tiles = []
            for hh in range(2):
                t = wpool.tile([128, D], F32R, tag=f"wo{hh}", name=f"wo{hh}")
                nc.sync.dma_start(t[:], woutT[hh * 128:(hh + 1) * 128, :])
                wo_tiles.append(t)
            mask_t = wpool.tile([128, NQT * QC], F32R, tag="mask", name="mask")
            nc.sync.dma_start(mask_t[:], masks[:])
            onec_t = wpool.tile([128, 1], F32R, tag="onec", name="onec")
            nc.sync.dma_start(onec_t[:], ones_col[:])
            oner_t = wpool.tile([1, 128], F32R, tag="oner", name="oner")
            nc.sync.dma_start(oner_t[:], ones_row[:])

            for b in range(B):
                # ---- persistent per-batch QKV / A tiles ----
                # comps: 0=Q_h0 1=K_h0 2=Q_h1 3=K_h1 (dh-major [128, S])
                qk_sb = [qkvpool.tile([128, S], F32R, tag=f"qk{c}", name=f"qk{c}")
                         for c in range(4)]
                # V token-major: tile per 128 tokens, [128, 256] (2 heads)
                v_sb = [qkvpool.tile([128, 256], F32R, tag=f"v{t}", name=f"v{t}")
                        for t in range(S // 128)]
                # A^T per head [128, S]
                a_sb = [apool.tile([128, S], F32R, tag=f"a{h}", name=f"a{h}")
                        for h in range(2)]

                # ======== Phase A: QKV projection for this batch ========
                for j in range(NCH):
                    col0 = b * S + j * QC
                    xt = []
                    for k in range(KT):
                        t = xpool.tile([128, QC], F32R, tag="xt", name="xt")
                        nc.sync.dma_start(
                            t[:], xT[k * 128:(k + 1) * 128, col0:col0 + QC])
                        xt.append(t)
                    # Q^T / K^T for both heads (copies on ScalarE: idle in
                    # this phase, keeps DVE free)
                    for c in range(4):
                        ps = pspool.tile([128, QC], F32, tag="ps", name="ps")
                        for k in range(KT):
                            nc.tensor.matmul(
                                ps[:],
                                (w_tiles[k][:, c * 128:(c + 1) * 128]),
                                (xt[k][:]),
                                start=(k == 0), stop=(k == KT - 1))
                        nc.scalar.copy(
                            qk_sb[c][:, j * QC:(j + 1) * QC], ps[:])
                    # V token-major (both heads side by side)
                    for sub in range(NQT):
                        ps = pspool.tile([128, 256], F32, tag="ps", name="ps")
                        for k in range(KT):
                            nc.tensor.matmul(
                                ps[:],
                                (xt[k][:, sub * 128:(sub + 1) * 128]),
                                (w_tiles[k][:, 512:768]),
                                start=(k == 0), stop=(k == KT - 1))
                        nc.scalar.copy(v_sb[j * NQT + sub][:], ps[:])

                # ======== Phase B+C: attention + out-proj per chunk ======
                # Normalization is software-pipelined one block behind so
                # the (slow) reciprocal never sits on the PE's in-order
                # path: block k's rank-1 broadcast + final mul are emitted
                # after block k+1's matmuls.

                def emit_attn_block(j, h):
                    n_kt = (j + 1) * QC // 128
                    qT = qk_sb[2 * h]
                    kTl = qk_sb[2 * h + 1]
                    attn = attnps.tile([128, QC], F32, tag="attn",
                                       name="attn")
                    r = rps.tile([1, QC], F32, tag="r", name="r")
                    for kt in range(n_kt):
                        s_ps = pspool.tile([128, QC], F32, tag="ps",
                                           name="ps")
                        nc.tensor.matmul(
                            s_ps[:],
                            kTl[:, kt * 128:(kt + 1) * 128],
                            qT[:, j * QC:(j + 1) * QC],
                            start=True, stop=True)
                        p_sb = ppool.tile([128, QC], F32R, tag="p", name="p")
                        nc.scalar.activation(
                            p_sb[:], s_ps[:],
                            mybir.ActivationFunctionType.Exp,
                            scale=inv_sqrt_dh)
                        rel = kt * 128 - j * QC
                        if rel >= 0:
                            # diagonal block: zero the k > q half
                            ridx = rel // 128
                            nc.vector.tensor_mul(
                                p_sb[:], p_sb[:],
                                mask_t[:, ridx * QC:(ridx + 1) * QC])
                        nc.tensor.matmul(
                            attn[:],
                            v_sb[kt][:, h * 128:(h + 1) * 128],
                            p_sb[:],
                            start=(kt == 0), stop=(kt == n_kt - 1))
                        nc.tensor.matmul(
                            r[:], onec_t[:], p_sb[:],
                            start=(kt == 0), stop=(kt == n_kt - 1))
                    # launch the reciprocal now (DVE), consumed one block
                    # later by the rank-1 broadcast
                    recip = smallpool.tile([1, QC], F32, tag="recip",
                                           name="recip")
                    nc.vector.reciprocal_approx_fast(recip[:], r[:])
                    recip_r = smallpool.tile([1, QC], F32R, tag="recipr",
                                             name="recipr")
                    nc.vector.tensor_copy(recip_r[:], recip[:])
                    return (j, h, attn, recip_r)

                def emit_finalize(blk):
                    j, h, attn, recip_r = blk
                    rb_ps = pspool.tile([128, QC], F32, tag="ps", name="ps")
                    nc.tensor.matmul(rb_ps[:], oner_t[:], recip_r[:],
                                     start=True, stop=True)
                    rb_sb = ppool.tile([128, QC], F32R, tag="p", name="p")
                    nc.vector.tensor_copy(rb_sb[:], rb_ps[:])
                    nc.vector.tensor_mul(
                        a_sb[h][:, j * QC:(j + 1) * QC], attn[:], rb_sb[:])

                def emit_outproj(j):
                    # partial over this core's 256 head-features
                    col0 = b * S + j * QC
                    for m in range(D // 128):
                        ps = pspool.tile([128, QC], F32, tag="ps", name="ps")
                        for h in range(2):
                            nc.tensor.matmul(
                                ps[:],
                                wo_tiles[h][:, m * 128:(m + 1) * 128],
                                a_sb[h][:, j * QC:(j + 1) * QC],
                                start=(h == 0), stop=(h == 1))
                        o_sb = opool.tile([128, QC], F32, tag="o", name="o")
                        nc.vector.tensor_copy(o_sb[:], ps[:])
                        nc.sync.dma_start(
                            outT[m * 128:(m + 1) * 128, col0:col0 + QC],
                            o_sb[:])

                pending = None
                for j in range(NCH):
                    for h in range(2):
                        blk = emit_attn_block(j, h)
                        if pending is not None:
                            emit_finalize(pending)
                            if pending[1] == 1:
                                emit_outproj(pending[0])
                        pending = blk
                emit_finalize(pending)
                emit_outproj(pending[0])

    nc.compile()
    return nc


def make_inputs(cfg: Cfg, x, W_qkv, W_out):
    """Host-side sharding: returns in_maps (list of 8 dicts)."""
    B, S, D = cfg.B, cfg.S, cfg.D
    Dh, QC, NQT = cfg.Dh, cfg.QC, cfg.QC // 128
    xTa = np.ascontiguousarray(
        x.reshape(B * S, D).T.astype(np.float32))          # [D, B*S]

    masks = np.zeros((128, NQT * QC), dtype=np.float32)
    for ridx in range(NQT):
        rel = ridx * 128
        p = np.arange(128)[:, None]
        f = np.arange(QC)[None, :]
        masks[:, ridx * QC:(ridx + 1) * QC] = (p + rel <= f).astype(np.float32)
    ones_col = np.ones((128, 1), dtype=np.float32)
    ones_row = np.ones((1, 128), dtype=np.float32)

    in_maps = []
    DHT = cfg.DHT
    for c in range(N_CORES):
        h0 = cfg.HPC * c
        wq = np.empty((D, 768), dtype=np.float32)          # [D, cols]
        for i, h in enumerate((h0, h0 + 1)):
            wq[:, (2 * i) * 128:(2 * i) * 128 + 128] = \
                W_qkv[0 * DHT + h * Dh: 0 * DHT + h * Dh + Dh, :].T   # Q_h
            wq[:, (2 * i + 1) * 128:(2 * i + 1) * 128 + 128] = \
                W_qkv[1 * DHT + h * Dh: 1 * DHT + h * Dh + Dh, :].T   # K_h
            wq[:, 512 + i * 128: 512 + (i + 1) * 128] = \
                W_qkv[2 * DHT + h * Dh: 2 * DHT + h * Dh + Dh, :].T   # V_h
        wo = np.ascontiguousarray(
            W_out[:, h0 * Dh:(h0 + cfg.HPC) * Dh].T.astype(np.float32))
        in_maps.append({
            "xT": xTa,
            "wqkvT": np.ascontiguousarray(wq),
            "woutT": wo,
            "masks": masks,
            "ones_col": ones_col,
            "ones_row": ones_row,
        })
    return in_maps


_CACHED = {}


def kernel(x, W_qkv, W_out, mask=None, **_ignored):
    cfg = Cfg(B=x.shape[0], S=x.shape[1], D=x.shape[2],
              n_heads=W_qkv.shape[0] // 384)
    key = (cfg.B, cfg.S, cfg.D)
    if key not in _CACHED:
        _CACHED[key] = build_kernel(cfg)
    nc = _CACHED[key]
    in_maps = make_inputs(cfg, np.asarray(x), np.asarray(W_qkv),
                          np.asarray(W_out))
    res = run_bass_kernel_spmd(nc, in_maps, list(range(N_CORES)))
    acc = res.results[0]["outT"].astype(np.float32)
    for c in range(1, N_CORES):
        acc = acc + res.results[c]["outT"]
    out = acc.T.reshape(cfg.B, cfg.S, cfg.D)
    return np.ascontiguousarray(out)


# revision 13
# speedup vs baseline: 1.2097x; 1.0124x over previous
"""Multi-head causal self-attention (B=2, S=2048, D=2048, H=16) on 8 trn2
NeuronCores.

Sharding: tensor-parallel over heads. Core c owns heads {2c, 2c+1}:
  - QKV projection for its 2 heads (contraction over the full d_model),
  - causal attention for its 2 heads,
  - partial output projection  O_c = A_c @ W_out[:, c*256:(c+1)*256].T
Host sums the 8 partial outputs (the "all-reduce after out_proj" of the
TP scheme, done on host since the full output is assembled there anyway).

All on-device compute is laid out "feature-major" (transposed) so no
transposes are ever needed:
  - x is shipped as xT [D, B*S]
  - Q^T, K^T per head as [Dh=128, S];  V token-major as [S, Dh] tiles
  - scores are built transposed: S^T[k, q] = (K Q^T)[k, q]
  - softmax without max-subtraction (scores are O(+-5)), with the
    normalizer computed by a ones-row matmul over partition (key) dim and
    applied via a rank-1 PE broadcast.
  - attention output lands as A^T [Dh, S]; out-proj consumes it directly.

Matmuls run as float32r (full PE rate at free-dim >= 256, fp32 storage).
"""

import math

import numpy as np

import concourse.bass as bass
import concourse.tile as tile
from concourse import bacc, mybir
from concourse.bass_utils import run_bass_kernel_spmd

F32 = mybir.dt.float32
F32R = mybir.dt.float32r

N_CORES = 8


class Cfg:
    def __init__(self, B=2, S=2048, D=2048, n_heads=16):
        self.B = B
        self.S = S
        self.D = D
        self.n_heads = n_heads
        self.Dh = 128
        self.DHT = n_heads * self.Dh       # W_qkv section stride (q/k/v)
        self.HPC = n_heads // N_CORES      # heads per core (2)
        self.QC = 512                      # token chunk (matmul free dim)
        self.KT = D // 128                 # k-tiles over d_model
        self.NCH = S // self.QC            # token chunks per batch
        assert self.HPC == 2 and D % 128 == 0 and S % self.QC == 0


def build_kernel(cfg: Cfg):
    """Build the SPMD single-core program. Returns compiled nc."""
    B, S, D, QC, KT, NCH = cfg.B, cfg.S, cfg.D, cfg.QC, cfg.KT, cfg.NCH
    Dh = cfg.Dh
    NQT = QC // 128                      # 128-token subtiles per chunk
    inv_sqrt_dh = 1.0 / math.sqrt(Dh)

    nc = bacc.Bacc("TRN2", target_bir_lowering=False, debug=False,
                   num_devices=N_CORES)

    xT = nc.dram_tensor("xT", [D, B * S], F32R, kind="ExternalInput").ap()
    wqkvT = nc.dram_tensor("wqkvT", [D, 768], F32R, kind="ExternalInput").ap()
    woutT = nc.dram_tensor("woutT", [256, D], F32R, kind="ExternalInput").ap()
    masks = nc.dram_tensor("masks", [128, NQT * QC], F32R,
                           kind="ExternalInput").ap()
    ones_col = nc.dram_tensor("ones_col", [128, 1], F32R,
                              kind="ExternalInput").ap()
    ones_row = nc.dram_tensor("ones_row", [1, 128], F32R,
                              kind="ExternalInput").ap()
    outT = nc.dram_tensor("outT", [D, B * S], F32, kind="ExternalOutput").ap()


    with tile.TileContext(nc) as tc:
        with (
            tc.tile_pool(name="wpool", bufs=1) as wpool,
            tc.tile_pool(name="xpool", bufs=5) as xpool,
            tc.tile_pool(name="qkvpool", bufs=1) as qkvpool,
            tc.tile_pool(name="apool", bufs=1) as apool,
            tc.tile_pool(name="ppool", bufs=5) as ppool,
            tc.tile_pool(name="opool", bufs=3) as opool,
            tc.tile_pool(name="smallpool", bufs=2) as smallpool,
            tc.tile_pool(name="pspool", bufs=4, space="PSUM") as pspool,
            tc.tile_pool(name="attnps", bufs=2, space="PSUM") as attnps,
            tc.tile_pool(name="rps", bufs=2, space="PSUM") as rps,
        ):
            # ---- static weights / constants ----
            w_tiles = []
            for k in range(KT):
                t = wpool.tile([128, 768], F32R, tag=f"w{k}", name=f"w{k}")
                nc.sync.dma_start(t[:], wqkvT[k * 128:(k + 1) * 128, :])
                w_tiles.append(t)
            wo_tiles = []
            for hh in range(2):
                t = wpool.tile([128, D], F32R, tag=f"wo{hh}", name=f"wo{hh}")
                nc.sync.dma_start(t[:], woutT[hh * 128:(hh + 1) * 128, :])
                wo_tiles.append(t)
            mask_t = wpool.tile([128, NQT * QC], F32R, tag="mask", name="mask")
            nc.sync.dma_start(mask_t[:], masks[:])
            onec_t = wpool.tile([128, 1], F32R, tag="onec", name="onec")
            nc.sync.dma_start(onec_t[:], ones_col[:])
            oner_t = wpool.tile([1, 128], F32R, tag="oner", name="oner")
            nc.sync.dma_start(oner_t[:], ones_row[:])

            for b in range(B):
                # ---- persistent per-batch QKV / A tiles ----
                # comps: 0=Q_h0 1=K_h0 2=Q_h1 3=K_h1 (dh-major [128, S])
                qk_sb = [qkvpool.tile([128, S], F32R, tag=f"qk{c}", name=f"qk{c}")
                         for c in range(4)]
                # V token-major: tile per 128 tokens, [128, 256] (2 heads)
                v_sb = [qkvpool.tile([128, 256], F32R, tag=f"v{t}", name=f"v{t}")
                        for t in range(S // 128)]
                # A^T per head [128, S]
                a_sb = [apool.tile([128, S], F32R, tag=f"a{h}", name=f"a{h}")
                        for h in range(2)]

                # ======== Phase A: QKV projection for this batch ========
                HKT = KT // 4
                for j in range(NCH):
                    col0 = b * S + j * QC
                    # four batched DMAs per chunk: [128, HKT*QC] quarters
                    # with the d_model k-tiles laid out along the free dim
                    halves = []
                    for hh in range(4):
                        t = xpool.tile([128, HKT * QC], F32R, tag="xt",
                                       name="xt")
                        src = xT[hh * HKT * 128:(hh + 1) * HKT * 128,
                                 col0:col0 + QC]
                        nc.sync.dma_start(
                            t[:].rearrange("p (k c) -> p k c", k=HKT),
                            src.rearrange("(k p) c -> p k c", p=128))
                        halves.append(t)

                    def xt_sl(k, f0, f1):
                        t = halves[k // HKT]
                        kk = k % HKT
                        return t[:, kk * QC + f0: kk * QC + f1]

                    # Q^T / K^T for both heads (copies on ScalarE: idle in
                    # this phase, keeps DVE free)
                    for c in range(4):
                        ps = pspool.tile([128, QC], F32, tag="ps", name="ps")
                        for k in range(KT):
                            nc.tensor.matmul(
                                ps[:],
                                (w_tiles[k][:, c * 128:(c + 1) * 128]),
                                (xt_sl(k, 0, QC)),
                                start=(k == 0), stop=(k == KT - 1))
                        nc.scalar.copy(
                            qk_sb[c][:, j * QC:(j + 1) * QC], ps[:])
                    # V token-major (both heads side by side)
                    for sub in range(NQT):
                        ps = pspool.tile([128, 256], F32, tag="ps", name="ps")
                        for k in range(KT):
                            nc.tensor.matmul(
                                ps[:],
                                (xt_sl(k, sub * 128, (sub + 1) * 128)),
                                (w_tiles[k][:, 512:768]),
                                start=(k == 0), stop=(k == KT - 1))
                        nc.scalar.copy(v_sb[j * NQT + sub][:], ps[:])

                # ======== Phase B+C: attention + out-proj per chunk ======
                # Normalization is software-pipelined one block behind so
                # the (slow) reciprocal never sits on the PE's in-order
                # path: block k's rank-1 broadcast + final mul are emitted
                # after block k+1's matmuls.

                def emit_attn_block(j, h):
                    # attnV/r matmuls lag the scores by SKEW k-tiles so the
                    # exp -> mask chain latency stays off the PE's in-order
                    # path.
                    SKEW = 2
                    n_kt = (j + 1) * QC // 128
                    qT = qk_sb[2 * h]
                    kTl = qk_sb[2 * h + 1]
                    attn = attnps.tile([128, QC], F32, tag="attn",
                                       name="attn")
                    r = rps.tile([1, QC], F32, tag="r", name="r")
                    p_tiles = {}

                    def emit_scores(kt):
                        s_ps = pspool.tile([128, QC], F32, tag="ps",
                                           name="ps")
                        nc.tensor.matmul(
                            s_ps[:],
                            kTl[:, kt * 128:(kt + 1) * 128],
                            qT[:, j * QC:(j + 1) * QC],
                            start=True, stop=True)
                        p_sb = ppool.tile([128, QC], F32R, tag="p", name="p")
                        nc.scalar.activation(
                            p_sb[:], s_ps[:],
                            mybir.ActivationFunctionType.Exp,
                            scale=inv_sqrt_dh)
                        rel = kt * 128 - j * QC
                        if rel >= 0:
                            # diagonal block: zero the k > q half
                            ridx = rel // 128
                            nc.vector.tensor_mul(
                                p_sb[:], p_sb[:],
                                mask_t[:, ridx * QC:(ridx + 1) * QC])
                        p_tiles[kt] = p_sb

                    def emit_av(kt):
                        p_sb = p_tiles.pop(kt)
                        nc.tensor.matmul(
                            attn[:],
                            v_sb[kt][:, h * 128:(h + 1) * 128],
                            p_sb[:],
                            start=(kt == 0), stop=(kt == n_kt - 1))
                        nc.tensor.matmul(
                            r[:], onec_t[:], p_sb[:],
                            start=(kt == 0), stop=(kt == n_kt - 1))

                    for kt in range(n_kt):
                        emit_scores(kt)
                        if kt >= SKEW:
                            emit_av(kt - SKEW)
                    for kt in range(max(0, n_kt - SKEW), n_kt):
                        emit_av(kt)
                    # launch the reciprocal now (DVE), consumed one block
                    # later by the rank-1 broadcast
                    recip = smallpool.tile([1, QC], F32, tag="recip",
                                           name="recip")
                    nc.vector.reciprocal_approx_fast(recip[:], r[:])
                    recip_r = smallpool.tile([1, QC], F32R, tag="recipr",
                                             name="recipr")
                    nc.vector.tensor_copy(recip_r[:], recip[:])
                    return (j, h, attn, recip_r)

                def emit_finalize(blk):
                    j, h, attn, recip_r = blk
                    rb_ps = pspool.tile([128, QC], F32, tag="ps", name="ps")
                    nc.tensor.matmul(rb_ps[:], oner_t[:], recip_r[:],
                                     start=True, stop=True)
                    rb_sb = ppool.tile([128, QC], F32R, tag="p", name="p")
                    nc.vector.tensor_copy(rb_sb[:], rb_ps[:])
                    nc.vector.tensor_mul(
                        a_sb[h][:, j * QC:(j + 1) * QC], attn[:], rb_sb[:])

                def emit_outproj(j):
                    # partial over this core's 256 head-features; psum
                    # drains alternate DVE/ACT so the PE never waits on a
                    # slot, and the output DMAs ride the idle GpSimd SWDGE.
                    col0 = b * S + j * QC
                    for m in range(D // 128):
                        ps = pspool.tile([128, QC], F32, tag="ps", name="ps")
                        for h in range(2):
                            nc.tensor.matmul(
                                ps[:],
                                wo_tiles[h][:, m * 128:(m + 1) * 128],
                                a_sb[h][:, j * QC:(j + 1) * QC],
                                start=(h == 0), stop=(h == 1))
                        o_sb = opool.tile([128, QC], F32, tag="o", name="o")
                        if m % 2 == 0:
                            nc.vector.tensor_copy(o_sb[:], ps[:])
                        else:
                            nc.scalar.copy(o_sb[:], ps[:])
                        nc.sync.dma_start(
                            outT[m * 128:(m + 1) * 128, col0:col0 + QC],
                            o_sb[:])

                pending = None
                for j in range(NCH):
                    for h in range(2):
                        blk = emit_attn_block(j, h)
                        if pending is not None:
                            emit_finalize(pending)
                            if pending[1] == 1:
                                emit_outproj(pending[0])
                        pending = blk
                emit_finalize(pending)
                emit_outproj(pending[0])

    nc.compile()
    return nc


def make_inputs(cfg: Cfg, x, W_qkv, W_out):
    """Host-side sharding: returns in_maps (list of 8 dicts)."""
    B, S, D = cfg.B, cfg.S, cfg.D
    Dh, QC, NQT = cfg.Dh, cfg.QC, cfg.QC // 128
    xTa = np.ascontiguousarray(
        x.reshape(B * S, D).T.astype(np.float32))          # [D, B*S]

    masks = np.zeros((128, NQT * QC), dtype=np.float32)
    for ridx in range(NQT):
        rel = ridx * 128
        p = np.arange(128)[:, None]
        f = np.arange(QC)[None, :]
        masks[:, ridx * QC:(ridx + 1) * QC] = (p + rel <= f).astype(np.float32)
    ones_col = np.ones((128, 1), dtype=np.float32)
    ones_row = np.ones((1, 128), dtype=np.float32)

    in_maps = []
    DHT = cfg.DHT
    for c in range(N_CORES):
        h0 = cfg.HPC * c
        wq = np.empty((D, 768), dtype=np.float32)          # [D, cols]
        for i, h in enumerate((h0, h0 + 1)):
            wq[:, (2 * i) * 128:(2 * i) * 128 + 128] = \
                W_qkv[0 * DHT + h * Dh: 0 * DHT + h * Dh + Dh, :].T   # Q_h
            wq[:, (2 * i + 1) * 128:(2 * i + 1) * 128 + 128] = \
                W_qkv[1 * DHT + h * Dh: 1 * DHT + h * Dh + Dh, :].T   # K_h
            wq[:, 512 + i * 128: 512 + (i + 1) * 128] = \
                W_qkv[2 * DHT + h * Dh: 2 * DHT + h * Dh + Dh, :].T   # V_h
        wo = np.ascontiguousarray(
            W_out[:, h0 * Dh:(h0 + cfg.HPC) * Dh].T.astype(np.float32))
        in_maps.append({
            "xT": xTa,
            "wqkvT": np.ascontiguousarray(wq),
            "woutT": wo,
            "masks": masks,
            "ones_col": ones_col,
            "ones_row": ones_row,
        })
    return in_maps


_CACHED = {}


def kernel(x, W_qkv, W_out, mask=None, **_ignored):
    cfg = Cfg(B=x.shape[0], S=x.shape[1], D=x.shape[2],
              n_heads=W_qkv.shape[0] // 384)
    key = (cfg.B, cfg.S, cfg.D)
    if key not in _CACHED:
        _CACHED[key] = build_kernel(cfg)
    nc = _CACHED[key]
    in_maps = make_inputs(cfg, np.asarray(x), np.asarray(W_qkv),
                          np.asarray(W_out))
    res = run_bass_kernel_spmd(nc, in_maps, list(range(N_CORES)))
    acc = res.results[0]["outT"].astype(np.float32)
    for c in range(1, N_CORES):
        acc = acc + res.results[c]["outT"]
    out = acc.T.reshape(cfg.B, cfg.S, cfg.D)
    return np.ascontiguousarray(out)
